# revision 1
# baseline (speedup 1.0000x reference)
"""GCN 2-layer message-passing kernel for 8 Trainium2 NeuronCores (Bass/Tile).

Math (reference):
    h1  = x @ W1.T + b1
    a1  = segment_sum(h1[src], dst)         # over E edges
    r1  = relu(a1)
    h2  = r1 @ W2.T + b2
    out = log_softmax(segment_sum(h2[src], dst))

Restructuring: by linearity,
    segment_sum(x @ W1.T + b1) = segment_sum(x) @ W1.T + deg * b1
so the first aggregation works on raw x rows and W1/b1 are applied per
128-destination window afterwards.

Sharding: destinations split across 8 cores (12500 rows each), edges sorted
by destination.  Windows are processed in PAIRS (one PSUM bank = 512 f32
columns) so stripe matmuls run at N=512 (phase 1) / N=128 (phase 2).
Per 128-dst window the first T edges of each dst form "stripes" aggregated
with identity-matmul PSUM accumulation (slot p -> dst p); overflow edges go
to "tail" chunks aggregated with on-chip generated one-hot matmuls
(is_equal against an iota tile).

Message staging: this environment's runtime image ships no bulk-gather
ucode (InstDMAGatherAnt needs the "mlp" Q7 library, excluded from the
bedrock image - issuing it wedges the device), and `indirect_dma_start`
is architecturally one descriptor per destination partition (128 rows /
~1.4us call measured on HW => ~4.7ms per phase at this scale).  So the
per-edge routing tables are applied on the host (pure byte staging from
the statically known graph): messages for each phase are laid out in
window/slot order as kernel inputs, and the device consumes them with
full-rate dense DMA.  All arithmetic (both segment-sum accumulations,
both GEMMs, biases, relu, log_softmax) runs on the NeuronCores.
"""

import math

import numpy as np
import ml_dtypes

import concourse.bacc as bacc
import concourse.mybir as mybir
from concourse.tile import TileContext
from concourse.bass_utils import run_bass_kernel_spmd
from concourse.masks import make_identity

BF16 = ml_dtypes.bfloat16
P = 128
NCORES = 8
T_STRIPES = 32


def _preprocess(edge_index, n_nodes):
    """Build per-core slot tables (paired-window column layout)."""
    npc = n_nodes // NCORES            # nodes per core
    nw = math.ceil(npc / P)            # windows per core
    assert nw % 2 == 0
    dpad = nw * P
    pad_idx = n_nodes                  # index of the zero row
    src = np.asarray(edge_index[0]).astype(np.int64)
    dst = np.asarray(edge_index[1]).astype(np.int64)
    core_of = dst // npc
    dstl_all = (dst - core_of * npc).astype(np.int32)

    stripes = []    # per core: [nw, P, T] int32 (src ids, pad_idx pads)
    tails = []      # per core: per window (src_cols [P, ntc], dstrel_cols)
    degs = []
    ntc_per = np.zeros((NCORES, nw), np.int64)

    for c in range(NCORES):
        m = core_of == c
        s_c = src[m].astype(np.int32)
        d_c = dstl_all[m]
        order = np.argsort(d_c, kind="stable")
        s_c = s_c[order]
        d_c = d_c[order]
        deg = np.bincount(d_c, minlength=dpad).astype(np.int64)
        starts = np.concatenate([[0], np.cumsum(deg)])
        rank = np.arange(len(d_c)) - starts[d_c]

        stripe = np.full((dpad, T_STRIPES), pad_idx, np.int32)
        mm = rank < T_STRIPES
        stripe[d_c[mm], rank[mm]] = s_c[mm]
        stripes.append(stripe.reshape(nw, P, T_STRIPES))

        tm = ~mm
        ts_, td = s_c[tm], d_c[tm]
        twin = td // P
        tdr = (td % P).astype(np.int32)
        tcnt = np.bincount(twin, minlength=nw)
        tstart = np.concatenate([[0], np.cumsum(tcnt)])
        per_win = []
        for w in range(nw):
            a, b = tstart[w], tstart[w + 1]
            n = b - a
            ntc = math.ceil(n / P) if n else 0
            ntc_per[c, w] = ntc
            sc = np.full((P, ntc), pad_idx, np.int32)
            dr = np.zeros((P, ntc), np.int32)
            if n:
                flat_i = np.arange(n)
                sc[flat_i % P, flat_i // P] = ts_[a:b]
                dr[flat_i % P, flat_i // P] = tdr[a:b]
            per_win.append((sc, dr))
        tails.append(per_win)
        degs.append(deg)

    ntc_w = ntc_per.max(axis=0).astype(np.int64)      # uniform across cores
    # columns per window PAIR: 2*T interleaved stripes, then both tails
    np_pairs = nw // 2
    ntc_pair = [(int(ntc_w[2 * q]), int(ntc_w[2 * q + 1]))
                for q in range(np_pairs)]
    CA = int(2 * T_STRIPES * np_pairs + ntc_w.sum())
    offsA = np.zeros((NCORES, P, CA), np.int32)
    dstrel = np.zeros((NCORES, P, int(ntc_w.sum())), np.int32)
    for c in range(NCORES):
        colA = 0
        colT = 0
        for q in range(np_pairs):
            we, wo = 2 * q, 2 * q + 1
            # interleaved stripes: [we_s0, wo_s0, we_s1, wo_s1, ...]
            offsA[c, :, colA:colA + 2 * T_STRIPES:2] = stripes[c][we]
            offsA[c, :, colA + 1:colA + 2 * T_STRIPES:2] = stripes[c][wo]
            colA += 2 * T_STRIPES
            for w in (we, wo):
                sc, dr = tails[c][w]
                k = int(ntc_w[w])
                blockA = np.full((P, k), pad_idx, np.int32)
                blockD = np.zeros((P, k), np.int32)
                blockA[:, :sc.shape[1]] = sc
                blockD[:, :dr.shape[1]] = dr
                offsA[c, :, colA:colA + k] = blockA
                dstrel[c, :, colT:colT + k] = blockD
                colA += k
                colT += k

    deg_arr = np.stack(degs).astype(BF16)             # [NCORES, dpad]
    return dict(
        npc=npc, nw=nw, dpad=dpad, pad_idx=pad_idx,
        ntc_w=ntc_w, ntc_pair=ntc_pair, CA=CA, offsA=offsA,
        dstrel=dstrel, deg=deg_arr,
    )


def _build_p1(in_c, hid_c, out_c, nw, npc, ntc_pair, CA, CT):
    """Launch 1: windowed segsum(x) + W1/b1 + relu + W2/b2 -> h2 (bf16)."""
    nc = bacc.Bacc("TRN2", target_bir_lowering=False, debug=False,
                   num_devices=NCORES)
    dt = mybir.dt
    dpad = nw * P
    W = 2 * in_c            # paired acc width (512 f32 = 1 PSUM bank)

    msgs_d = nc.dram_tensor("msgsA", [P, CA * in_c], dt.bfloat16,
                            kind="ExternalInput")
    if CT:
        dstrel_d = nc.dram_tensor("dstrel", [P, CT], dt.int32,
                                  kind="ExternalInput")
    deg_d = nc.dram_tensor("deg", [1, dpad], dt.bfloat16, kind="ExternalInput")
    W1t_d = nc.dram_tensor("W1t", [in_c, hid_c], dt.bfloat16,
                           kind="ExternalInput")
    W2t_d = nc.dram_tensor("W2t", [hid_c, out_c], dt.bfloat16,
                           kind="ExternalInput")
    b1r_d = nc.dram_tensor("b1r", [1, hid_c], dt.bfloat16,
                           kind="ExternalInput")
    b2c_d = nc.dram_tensor("b2c", [out_c, 1], dt.float32,
                           kind="ExternalInput")
    h2_d = nc.dram_tensor("h2", [dpad, out_c], dt.bfloat16,
                          kind="ExternalOutput")

    n_in_k = in_c // P      # 2
    n_hid_m = hid_c // P    # 2

    with TileContext(nc) as tc:
        with (
            tc.tile_pool(name="const", bufs=1) as cpool,
            tc.tile_pool(name="io", bufs=4) as iop,
            tc.tile_pool(name="work", bufs=3) as wp,
            tc.tile_pool(name="psA", bufs=3, space="PSUM") as psA,
            tc.tile_pool(name="psE", bufs=1, space="PSUM") as psE,
        ):
            identb = cpool.tile([P, P], dt.bfloat16, tag="ident")
            make_identity(nc, identb[:])
            iota = cpool.tile([P, P], dt.int32, tag="iota")
            nc.gpsimd.iota(iota[:], pattern=[[1, P]], base=0,
                           channel_multiplier=0)
            w1 = cpool.tile([P, n_in_k * hid_c], dt.bfloat16, tag="w1")
            for k in range(n_in_k):
                nc.sync.dma_start(out=w1[:, k * hid_c:(k + 1) * hid_c],
                                  in_=W1t_d[k * P:(k + 1) * P, :])
            w2 = cpool.tile([P, n_hid_m * out_c], dt.bfloat16, tag="w2")
            for k in range(n_hid_m):
                nc.sync.dma_start(out=w2[:, k * out_c:(k + 1) * out_c],
                                  in_=W2t_d[k * P:(k + 1) * P, :])
            b1r = cpool.tile([1, hid_c], dt.bfloat16, tag="b1r")
            nc.sync.dma_start(out=b1r[:], in_=b1r_d[:])
            b2c = cpool.tile([out_c, 1], dt.float32, tag="b2c")
            nc.sync.dma_start(out=b2c[:], in_=b2c_d[:])
            degsb = cpool.tile([1, dpad], dt.bfloat16, tag="deg")
            nc.sync.dma_start(out=degsb[:], in_=deg_d[:])
            if CT:
                drel_all = cpool.tile([P, CT], dt.int32, tag="drelall")
                nc.sync.dma_start(out=drel_all[:], in_=dstrel_d[:])
            h2all = cpool.tile([P, nw * out_c], dt.bfloat16, tag="h2all")

            colA = 0
            colT = 0
            for q in range(nw // 2):
                ntc_e, ntc_o = ntc_pair[q]
                S = 2 * T_STRIPES + ntc_e + ntc_o
                w0 = 2 * q

                g = iop.tile([P, S * in_c], dt.bfloat16, tag="gA")
                eng = nc.sync if (q % 2 == 0) else nc.scalar
                eng.dma_start(
                    out=g[:], in_=msgs_d[:, colA * in_c:(colA + S) * in_c])
                ntc = ntc_e + ntc_o

                acc = psA.tile([P, W], dt.float32, tag="acc")
                for s in range(T_STRIPES):
                    nc.tensor.matmul(
                        out=acc[:], lhsT=identb[:],
                        rhs=g[:, 2 * s * in_c:2 * (s + 1) * in_c],
                        start=(s == 0), stop=(s == T_STRIPES - 1 and ntc == 0),
                    )
                for t in range(ntc):
                    half = 0 if t < ntc_e else 1
                    oh = wp.tile([P, P], dt.bfloat16, tag="ohA")
                    nc.vector.tensor_tensor(
                        out=oh[:],
                        in0=drel_all[:, colT + t:colT + t + 1]
                            .to_broadcast([P, P]),
                        in1=iota[:], op=mybir.AluOpType.is_equal,
                    )
                    nc.tensor.matmul(
                        out=acc[:, half * in_c:(half + 1) * in_c],
                        lhsT=oh[:],
                        rhs=g[:, (2 * T_STRIPES + t) * in_c:
                              (2 * T_STRIPES + t + 1) * in_c],
                        start=False, stop=(t == ntc - 1),
                    )

                agg_sb = wp.tile([P, W], dt.bfloat16, tag="aggsb")
                nc.vector.tensor_copy(out=agg_sb[:], in_=acc[:])
                # transpose to channel-major; layout [k0: we|wo, k1: we|wo]
                aggT_ps = psE.tile([P, 2 * n_in_k * P], dt.bfloat16,
                                   tag="aggT")
                for half in range(2):
                    for k in range(n_in_k):
                        nc.tensor.transpose(
                            out=aggT_ps[:, k * 2 * P + half * P:
                                        k * 2 * P + (half + 1) * P],
                            in_=agg_sb[:, half * in_c + k * P:
                                       half * in_c + (k + 1) * P],
                            identity=identb[:],
                        )
                aggT = wp.tile([P, 2 * n_in_k * P], dt.bfloat16, tag="aggTsb")
                nc.vector.tensor_copy(out=aggT[:], in_=aggT_ps[:])

                # GEMM1: h1 [hid chunk m (128), 2P dst] per m; + b1 x deg
                h1_ps = psE.tile([P, n_hid_m * 2 * P], dt.float32, tag="h1")
                for m_ in range(n_hid_m):
                    for k in range(n_in_k):
                        nc.tensor.matmul(
                            out=h1_ps[:, m_ * 2 * P:(m_ + 1) * 2 * P],
                            lhsT=w1[:, k * hid_c + m_ * P:
                                    k * hid_c + (m_ + 1) * P],
                            rhs=aggT[:, k * 2 * P:(k + 1) * 2 * P],
                            start=(k == 0), stop=False,
                        )
                    nc.tensor.matmul(
                        out=h1_ps[:, m_ * 2 * P:(m_ + 1) * 2 * P],
                        lhsT=b1r[:, m_ * P:(m_ + 1) * P],
                        rhs=degsb[:, w0 * P:(w0 + 2) * P],
                        start=False, stop=True,
                    )
                h1r = wp.tile([P, n_hid_m * 2 * P], dt.bfloat16, tag="h1r")
                for m_ in range(n_hid_m):
                    nc.scalar.activation(
                        out=h1r[:, m_ * 2 * P:(m_ + 1) * 2 * P],
                        in_=h1_ps[:, m_ * 2 * P:(m_ + 1) * 2 * P],
                        func=mybir.ActivationFunctionType.Relu,
                    )
                h2_ps = psE.tile([out_c, 2 * P], dt.float32, tag="h2t")
                for k in range(n_hid_m):
                    nc.tensor.matmul(
                        out=h2_ps[:],
                        lhsT=w2[:, k * out_c:(k + 1) * out_c],
                        rhs=h1r[:, k * 2 * P:(k + 1) * 2 * P],
                        start=(k == 0), stop=(k == n_hid_m - 1),
                    )
                h2t_sb = wp.tile([out_c, 2 * P], dt.bfloat16, tag="h2tsb")
                nc.vector.tensor_scalar(
                    out=h2t_sb[:], in0=h2_ps[:], scalar1=b2c[:], scalar2=None,
                    op0=mybir.AluOpType.add,
                )
                h2nm_ps = psE.tile([P, 2 * out_c], dt.bfloat16, tag="h2nm")
                for half in range(2):
                    nc.tensor.transpose(
                        out=h2nm_ps[:, half * out_c:(half + 1) * out_c],
                        in_=h2t_sb[:, half * P:(half + 1) * P],
                        identity=identb[0:out_c, 0:out_c])
                nc.vector.tensor_copy(
                    out=h2all[:, w0 * out_c:(w0 + 2) * out_c],
                    in_=h2nm_ps[:])
                colA += S
                colT += ntc

            nc.sync.dma_start(
                out=h2_d[:].rearrange("(w p) f -> p w f", p=P),
                in_=h2all[:].rearrange("p (w f) -> p w f", f=out_c))

    nc.compile()
    return nc


def _build_p2(out_c, nw, npc, ntc_pair, CA, CT):
    """Launch 2: windowed segsum(h2) + log_softmax (batched Ln)."""
    nc = bacc.Bacc("TRN2", target_bir_lowering=False, debug=False,
                   num_devices=NCORES)
    dt = mybir.dt
    W = 2 * out_c
    dpad = nw * P

    msgs_d = nc.dram_tensor("msgsB", [P, CA * out_c], dt.bfloat16,
                            kind="ExternalInput")
    if CT:
        dstrel_d = nc.dram_tensor("dstrel", [P, CT], dt.int32,
                                  kind="ExternalInput")
    out_d = nc.dram_tensor("out", [dpad, out_c], dt.float32,
                           kind="ExternalOutput")

    with TileContext(nc) as tc:
        with (
            tc.tile_pool(name="const", bufs=1) as cpool,
            tc.tile_pool(name="io", bufs=3) as iop,
            tc.tile_pool(name="work", bufs=2) as wp,
            tc.tile_pool(name="psB", bufs=2, space="PSUM") as psB,
        ):
            identb = cpool.tile([P, P], dt.bfloat16, tag="ident")
            make_identity(nc, identb[:])
            iota = cpool.tile([P, P], dt.int32, tag="iota")
            nc.gpsimd.iota(iota[:], pattern=[[1, P]], base=0,
                           channel_multiplier=0)
            # persistent stashes for the batched log path
            xm_all = cpool.tile([P, nw * out_c], dt.bfloat16, tag="xmall")
            sm_all = cpool.tile([P, nw], dt.float32, tag="small")
            lg_all = cpool.tile([P, nw], dt.float32, tag="lgall")
            ls_all = cpool.tile([P, nw * out_c], dt.float32, tag="lsall")
            if CT:
                drel_all = cpool.tile([P, CT], dt.int32, tag="drelall")
                nc.sync.dma_start(out=drel_all[:], in_=dstrel_d[:])

            colA = 0
            colT = 0
            for q in range(nw // 2):
                ntc_e, ntc_o = ntc_pair[q]
                S = 2 * T_STRIPES + ntc_e + ntc_o
                w0 = 2 * q

                g = iop.tile([P, S * out_c], dt.bfloat16, tag="gB")
                nc.sync.dma_start(
                    out=g[:], in_=msgs_d[:, colA * out_c:(colA + S) * out_c])
                ntc = ntc_e + ntc_o

                acc = psB.tile([P, W], dt.float32, tag="acc")
                for s in range(T_STRIPES):
                    nc.tensor.matmul(
                        out=acc[:], lhsT=identb[:],
                        rhs=g[:, 2 * s * out_c:2 * (s + 1) * out_c],
                        start=(s == 0), stop=(s == T_STRIPES - 1 and ntc == 0),
                    )
                for t in range(ntc):
                    half = 0 if t < ntc_e else 1
                    oh = wp.tile([P, P], dt.bfloat16, tag="ohB")
                    nc.vector.tensor_tensor(
                        out=oh[:],
                        in0=drel_all[:, colT + t:colT + t + 1]
                            .to_broadcast([P, P]),
                        in1=iota[:], op=mybir.AluOpType.is_equal,
                    )
                    nc.tensor.matmul(
                        out=acc[:, half * out_c:(half + 1) * out_c],
                        lhsT=oh[:],
                        rhs=g[:, (2 * T_STRIPES + t) * out_c:
                              (2 * T_STRIPES + t + 1) * out_c],
                        start=False, stop=(t == ntc - 1),
                    )

                for half in range(2):
                    w = w0 + half
                    a = acc[:, half * out_c:(half + 1) * out_c]
                    mx = wp.tile([P, 1], dt.float32, tag="mx")
                    nc.vector.tensor_reduce(out=mx[:], in_=a,
                                            axis=mybir.AxisListType.X,
                                            op=mybir.AluOpType.max)
                    nc.vector.tensor_scalar(
                        out=xm_all[:, w * out_c:(w + 1) * out_c], in0=a,
                        scalar1=mx[:], scalar2=None,
                        op0=mybir.AluOpType.subtract,
                    )
                    ex = wp.tile([P, out_c], dt.float32, tag="ex")
                    nc.scalar.activation(
                        out=ex[:], in_=xm_all[:, w * out_c:(w + 1) * out_c],
                        func=mybir.ActivationFunctionType.Exp,
                        accum_out=sm_all[:, w:w + 1])
                colA += S
                colT += ntc

            # one Ln over all windows' sums, then final subtract + store
            nc.scalar.activation(out=lg_all[:], in_=sm_all[:],
                                 func=mybir.ActivationFunctionType.Ln)
            for w in range(nw):
                nc.vector.tensor_scalar(
                    out=ls_all[:, w * out_c:(w + 1) * out_c],
                    in0=xm_all[:, w * out_c:(w + 1) * out_c],
                    scalar1=lg_all[:, w:w + 1], scalar2=None,
                    op0=mybir.AluOpType.subtract,
                )
            nc.sync.dma_start(
                out=out_d[:].rearrange("(w p) f -> p w f", p=P),
                in_=ls_all[:].rearrange("p (w f) -> p w f", f=out_c))

    nc.compile()
    return nc


def _stage_messages(values_padded, offs_c):
    """values_padded [n+1, feat] (last row zero), offs_c [P, CA] ->
    [P, CA*feat] staged message array."""
    return values_padded[offs_c].reshape(P, -1)


def _run(nc, in_maps, trace=False):
    return run_bass_kernel_spmd(nc, in_maps, list(range(NCORES)), trace=trace)


def kernel(x, edge_index, W1, b1, W2, b2):
    x = np.asarray(x)
    n_nodes, in_c = x.shape
    hid_c = W1.shape[0]
    out_c = W2.shape[0]
    pre = _preprocess(edge_index, n_nodes)
    npc, nw, CA = pre["npc"], pre["nw"], pre["CA"]
    CT = int(pre["ntc_w"].sum())

    nc1 = _build_p1(in_c, hid_c, out_c, nw, npc, pre["ntc_pair"], CA, CT)
    nc2 = _build_p2(out_c, nw, npc, pre["ntc_pair"], CA, CT)

    x_bf = np.zeros((n_nodes + 1, in_c), BF16)
    x_bf[:n_nodes] = x.astype(np.float32)
    W1t = np.ascontiguousarray(np.asarray(W1, np.float32).T).astype(BF16)
    W2t = np.ascontiguousarray(np.asarray(W2, np.float32).T).astype(BF16)
    b1r = np.asarray(b1, np.float32).reshape(1, -1).astype(BF16)
    b2c = np.asarray(b2, np.float32).reshape(-1, 1)

    in_maps1 = []
    for c in range(NCORES):
        m = {
            "msgsA": _stage_messages(x_bf, pre["offsA"][c]),
            "deg": pre["deg"][c].reshape(1, -1),
            "W1t": W1t, "W2t": W2t, "b1r": b1r, "b2c": b2c,
        }
        if CT:
            m["dstrel"] = pre["dstrel"][c]
        in_maps1.append(m)
    res1 = _run(nc1, in_maps1)
    h2 = np.concatenate([res1.results[c]["h2"][:npc] for c in range(NCORES)],
                        axis=0)

    h2_pad = np.zeros((n_nodes + 1, out_c), BF16)
    h2_pad[:n_nodes] = h2
    in_maps2 = []
    for c in range(NCORES):
        m = {"msgsB": _stage_messages(h2_pad, pre["offsA"][c])}
        if CT:
            m["dstrel"] = pre["dstrel"][c]
        in_maps2.append(m)
    res2 = _run(nc2, in_maps2)
    out = np.concatenate([res2.results[c]["out"][:npc] for c in range(NCORES)],
                         axis=0)
    return out.astype(np.float32)

